# revision 28
# baseline (speedup 1.0000x reference)
"""Trainium2 Bass kernel for nn_AdvancedIQCNN.

Pipeline (per sample):
  h  = relu(bn(x @ W1.T + b1)) ; h = relu(bn(h @ W2.T + b2))   (BN over full batch)
  xq = quantum(h)                                              (13-qubit circuits)
  out = relu(xq@W3.T+b3) -> relu(@W4.T+b4) -> @W5.T+b5

The quantum layer is evaluated in closed form. Propagating the measured
observable P(qubit0=1) backward through the shallow circuits (Heisenberg
picture, CX-chain conjugation of the Pauli string) collapses the 2^13
statevector simulation to 6 terms built from sin/cos of h[:, 0:3]:

  xq = k0 + k1*cos(h0) + k2*sin(h0)sin(h1) + k3*sin(h0)sin(h2)
          + k4*cos(h0)sin(h1)sin(h2) + k5*cos(h1)

with k* precomputed from the (replicated, tiny) theta angles.

Sharding: pure data parallel over 8 cores. Each core receives the full x
batch-rotated so its own 512-sample shard sits in packed slot 0; every core
redundantly computes the (tiny) front MLP over the full batch to get exact
BatchNorm statistics without collectives, then runs the quantum closed form
and back MLP on its shard only.

Performance structure:
  - 4 batch chunks are packed along the partition dim with block-diagonal
    weights (K=4*13=52/4*26=104 <= 128), so one matmul and one evac/square/
    relu instruction process 4 chunks at once; the front MLP over the full
    4096-batch is 2 matmuls per layer instead of 8. Exact fp32 throughout
    (the block-diagonal zeros contribute exact 0.0 to the fp32 dot products).
  - BN statistics come from accum_out side channels of the PSUM-evacuation
    (sum) and an ACT Square pass (sum of squares); partition-packed partials
    are folded with tiny summing matmuls, and the per-feature scale/shift
    is broadcast back to the packed layout with a replication matmul.
  - quantum term rows are built with small selection matmuls on the PE.
  - one packed weights/consts DMA + one x DMA (2 chunks).
"""

import sys

if "/opt/trn_rl_repo" not in sys.path:
    sys.path.insert(0, "/opt/trn_rl_repo")

from contextlib import ExitStack

import numpy as np

B = 4096
NF = 13
NCORES = 8
SH = B // NCORES  # 512 samples per core
CH = 512
PK = 4            # chunks packed along partitions
NJ = B // (CH * PK)  # 2 column blocks
K1 = PK * NF      # 52
K2 = PK * 26      # 104

# wpack column layout ([K2=104] partitions x WCOLS fp32)
_C = {}
_o = 0


def _col(name, n):
    global _o
    _C[name] = (_o, _o + n)
    _o += n


_col("W1BD", K2)   # [52, 104] block-diag of W1.T [13,26] x4
_col("W2BD", K1)   # [104, 52] block-diag of W2.T [26,13] x4
_col("W3T", 32)    # [1, 32]
_col("W4T", 16)    # [32, 16]
_col("W5T", 2)     # [16, 2]
_col("KC", 1)      # [6, 1]
_col("B1R", 1)     # [104, 1] b1 tiled x4
_col("B2R", 1)     # [52, 1]  b2 tiled x4
_col("B3", 1)      # [32, 1]
_col("B4", 1)      # [16, 1]
_col("B5", 1)      # [2, 1]
_col("G1", 1)      # [26, 1]
_col("BE1", 1)
_col("G2", 1)      # [13, 1]
_col("BE2", 1)
_col("SUM1", 26)   # [104, 26] 4-stacked identity (sums partition groups)
_col("SUM2", NF)   # [52, 13]
_col("REP1", K2)   # [26, 104] replication (scale/shift -> packed)
_col("D36", 6)     # [3, 6] duplication selection
_col("S1", 6)      # [6, 6] M1 selection
_col("S2", 6)
_col("S3", 6)
_col("SINB", 1)    # [6, 1] sin biases [pi,pi,pi,pi/2,pi/2,pi/2]
_col("M1S", 1)     # [6, 1] evac scale/bias columns
_col("M1B", 1)
_col("M2S", 1)
_col("M2B", 1)
_col("M3S", 1)
_col("M3B", 1)
_col("EPS", 1)     # 1e-5
WCOLS = _o


def _build_nc(reps=1, loop_n=1):
    import concourse.bass as bass
    import concourse.mybir as mybir
    import concourse.tile as tile
    from concourse import bacc

    dt = mybir.dt.float32
    AF = mybir.ActivationFunctionType
    AL = mybir.AluOpType
    ts = bass.ts

    nc = bacc.Bacc("TRN2", target_bir_lowering=False, debug=False)

    xS = nc.dram_tensor("xS", [K1, NJ * CH], dt, kind="ExternalInput").ap()
    wp = nc.dram_tensor("wp", [K2, WCOLS], dt, kind="ExternalInput").ap()
    outT = nc.dram_tensor("outT", [2, SH], dt, kind="ExternalOutput").ap()

    with tile.TileContext(nc) as tc, ExitStack() as ctx:
        pool = ctx.enter_context(tc.tile_pool(name="sb", bufs=1))
        sqp = ctx.enter_context(tc.tile_pool(name="sq", bufs=2))
        psum = ctx.enter_context(tc.tile_pool(name="ps", bufs=7, space="PSUM"))

        for i, val in enumerate((0.0,)):
            t = pool.tile([128, 1], dt, tag=f"const{i}")
            nc.vector.memset(t[:], val)
            nc.const_aps.aps[(dt, val)] = t[:]

        # PE p-state warm-up: one long dummy matmul keeps the PE busy during
        # the input DMAs so the real matmuls run at full clock.
        wrm = pool.tile([1, CH + 1], dt, tag="wrm")
        nc.gpsimd.memset(wrm[:], 0.0)
        pwm = psum.tile([1, CH], dt, tag="warm", bufs=1)
        nc.tensor.matmul(pwm[:], wrm[0:1, 0:1], wrm[0:1, 1 : CH + 1])

        w = pool.tile([K2, WCOLS], dt, tag="wp")
        nc.sync.dma_start(out=w[:], in_=wp[:])

        def W(name, p):
            lo, hi = _C[name]
            return w[0:p, lo:hi]

        xsb = pool.tile([K1, NJ * CH], dt, tag="xsb")
        for j in range(NJ):
            nc.sync.dma_start(out=xsb[:, ts(j, CH)], in_=xS[:, ts(j, CH)])

        def mm(out_ap, lhsT, rhs, **kw):
            nc.tensor.matmul(out_ap, lhsT, rhs, **kw)

        def packed_bn_layer(in_sb, kin, wname, kout, fout, brname, sumname,
                            gname, bename, lname):
            """Packed z = blockdiag(wT).T@in + b; BN stats via accum_out.
            Returns (z tile [kout, NJ*CH], scale [fout,1], shift [fout,1])."""
            z = pool.tile([kout, NJ * CH], dt, tag=f"z{lname}")
            parts = pool.tile([kout, 2 * NJ], dt, tag=f"parts{lname}")
            bcol = W(brname, kout)
            for j in range(NJ):
                pm = psum.tile([kout, CH], dt, tag="mm")
                mm(pm[:], W(wname, kin), in_sb[:, ts(j, CH)])
                # evac + bias; accum -> per-packed-row sum partial (col j)
                nc.vector.tensor_scalar(
                    z[:, ts(j, CH)], pm[:], bcol, None, op0=AL.add, op1=AL.add,
                    accum_out=parts[:, j : j + 1],
                )
                # (z)^2 straight from PSUM; accum -> sumsq partial (col NJ+j)
                sq = sqp.tile([kout, CH], dt, tag="sqscr")
                nc.scalar.activation(
                    sq[:], pm[:], AF.Square, bias=bcol,
                    accum_out=parts[:, NJ + j : NJ + j + 1],
                )
            # fold the PK partition groups: [kout, 2*NJ] -> [fout, 2*NJ]
            pf = psum.tile([fout, 2 * NJ], dt, tag="mm")
            mm(pf[:], W(sumname, kout), parts[:])
            st = pool.tile([fout, 2 * NJ], dt, tag=f"st{lname}")
            nc.vector.tensor_scalar_add(st[:], pf[:], 0.0)
            # reduce the NJ column blocks: view [fout, 2, NJ] -> [fout, 2]
            tot = pool.tile([fout, 2], dt, tag=f"tot{lname}")
            nc.vector.reduce_sum(
                tot[:], st[:].rearrange("p (k j) -> p k j", k=2),
                axis=mybir.AxisListType.X,
            )
            mean = pool.tile([fout, 1], dt, tag=f"mean{lname}")
            nc.vector.tensor_scalar_mul(mean[:], tot[:, 0:1], 1.0 / B)
            m2 = pool.tile([fout, 1], dt, tag=f"m2{lname}")
            nc.vector.tensor_mul(m2[:], mean[:], mean[:])
            var = pool.tile([fout, 1], dt, tag=f"var{lname}")
            nc.vector.tensor_scalar(var[:], tot[:, 1:2], 1.0 / B, None, op0=AL.mult)
            nc.vector.tensor_sub(var[:], var[:], m2[:])
            std = pool.tile([fout, 1], dt, tag=f"std{lname}")
            nc.scalar.activation(std[:], var[:], AF.Sqrt, bias=W("EPS", fout))
            rstd = pool.tile([fout, 1], dt, tag=f"rstd{lname}")
            nc.vector.reciprocal(rstd[:], std[:])
            scale = pool.tile([fout, 1], dt, tag=f"scale{lname}")
            nc.vector.tensor_mul(scale[:], rstd[:], W(gname, fout))
            shift = pool.tile([fout, 1], dt, tag=f"shift{lname}")
            nc.vector.tensor_mul(shift[:], mean[:], scale[:])
            nc.vector.tensor_sub(shift[:], W(bename, fout), shift[:])
            return z, scale, shift

        def body():
            z1, sc1, sh1 = packed_bn_layer(
                xsb, K1, "W1BD", K2, 26, "B1R", "SUM1", "G1", "BE1", "1"
            )
            # replicate scale/shift back to the packed layout: [26,2] -> [104,2]
            ss26 = pool.tile([26, 2], dt, tag="ss26")
            nc.vector.tensor_copy(ss26[:, 0:1], sc1[:])
            nc.vector.tensor_copy(ss26[:, 1:2], sh1[:])
            pr = psum.tile([K2, 2], dt, tag="mm")
            mm(pr[:], W("REP1", 26), ss26[:])
            ssr = pool.tile([K2, 2], dt, tag="ssr")
            nc.vector.tensor_scalar_add(ssr[:], pr[:], 0.0)

            h1 = pool.tile([K2, NJ * CH], dt, tag="h1")
            for j in range(NJ):
                nc.scalar.activation(
                    h1[:, ts(j, CH)], z1[:, ts(j, CH)], AF.Relu,
                    bias=ssr[:, 1:2], scale=ssr[:, 0:1],
                )

            return packed_bn_layer(
                h1, K2, "W2BD", K1, NF, "B2R", "SUM2", "G2", "BE2", "2"
            )

        def tail(z2, sc2, sh2):
            # ---- quantum closed form on features 0..2 of the local shard ----
            # local shard = packed slot 0 = partitions 0:13 of column block 0
            hq = pool.tile([3, SH], dt, tag="hq")
            nc.scalar.activation(
                hq[:], z2[0:3, 0:SH], AF.Relu, bias=sh2[0:3, :], scale=sc2[0:3, :]
            )
            # duplicate to 6 rows; scc = sin(SINB - hq6) = [s0,s1,s2,c0,c1,c2]
            p6 = psum.tile([6, SH], dt, tag="mm")
            mm(p6[:], W("D36", 3), hq[:])
            scc = pool.tile([6, SH], dt, tag="scc")
            nc.scalar.activation(scc[:], p6[:], AF.Sin, bias=W("SINB", 6), scale=-1.0)

            # M1=[1,c0,c1,s0,s0,c0], M2=[1,1,1,s1,s2,s2], M3=[1,1,1,1,1,s1]
            Ms = []
            for sname, scl, bia in (("S1", "M1S", "M1B"), ("S2", "M2S", "M2B"),
                                    ("S3", "M3S", "M3B")):
                pm = psum.tile([6, SH], dt, tag="mm")
                mm(pm[:], W(sname, 6), scc[:])
                m = pool.tile([6, SH], dt, tag=f"m{sname}")
                nc.vector.tensor_scalar(
                    m[:], pm[:], W(scl, 6), W(bia, 6), op0=AL.mult, op1=AL.add
                )
                Ms.append(m)
            T = pool.tile([6, SH], dt, tag="T")
            nc.vector.tensor_mul(T[:], Ms[0][:], Ms[1][:])
            nc.vector.tensor_mul(T[:], T[:], Ms[2][:])

            xqp = psum.tile([1, SH], dt, tag="mm")
            mm(xqp[:], W("KC", 6), T[:])
            xq = pool.tile([1, SH], dt, tag="xq")
            nc.scalar.copy(xq[:], xqp[:])

            # ---- back MLP ----
            z3 = psum.tile([32, SH], dt, tag="mm")
            mm(z3[:], W("W3T", 1), xq[:])
            h3 = pool.tile([32, SH], dt, tag="h3")
            nc.scalar.activation(h3[:], z3[:], AF.Relu, bias=W("B3", 32))
            z4 = psum.tile([16, SH], dt, tag="mm")
            mm(z4[:], W("W4T", 32), h3[:])
            h4 = pool.tile([16, SH], dt, tag="h4")
            nc.scalar.activation(h4[:], z4[:], AF.Relu, bias=W("B4", 16))
            z5 = psum.tile([2, SH], dt, tag="mm")
            mm(z5[:], W("W5T", 16), h4[:])
            o = pool.tile([2, SH], dt, tag="o")
            nc.scalar.activation(o[:], z5[:], AF.Identity, bias=W("B5", 2))
            nc.sync.dma_start(out=outT[:], in_=o[:])

        if loop_n > 1:
            with tc.For_i(0, loop_n, 1):
                tail(*body())
        else:
            for _rep in range(reps):
                tail(*body())

    nc.compile()
    return nc


def _wpack(inputs):
    f32 = np.float32
    a, b, t = (
        np.asarray(inputs["th1a"], f32),
        np.asarray(inputs["th1b"], f32),
        np.asarray(inputs["th2a"], f32),
    )
    ca0, sa0 = np.cos(a[0]), np.sin(a[0])
    ca1, sa1 = np.cos(a[1]), np.sin(a[1])
    cb0, sb0 = np.cos(b[0]), np.sin(b[0])
    ct0, st0 = np.cos(t[0]), np.sin(t[0])
    # xq = 0.5 - (E1+E2)/4, T rows = [1, c0, c1, s0s1, s0s2, c0s1s2]
    kcv = np.array(
        [
            0.5,
            -(cb0 * ca0 + ct0) / 4.0,
            (sb0 * sa0 * sa1) / 4.0,
            (cb0 * sa0 + st0) / 4.0,
            (sb0 * ca0 * ca1) / 4.0,
            (sb0 * sa0 * ca1) / 4.0,
        ],
        f32,
    )

    wpk = np.zeros((K2, WCOLS), f32)

    def put(name, arr):
        lo, hi = _C[name]
        arr = np.asarray(arr, f32)
        if arr.ndim == 1:
            arr = arr[:, None]
        wpk[: arr.shape[0], lo:hi] = arr

    w1t = np.asarray(inputs["W1"], f32).T  # [13, 26]
    w2t = np.asarray(inputs["W2"], f32).T  # [26, 13]
    w1bd = np.zeros((K1, K2), f32)
    w2bd = np.zeros((K2, K1), f32)
    sum1 = np.zeros((K2, 26), f32)
    sum2 = np.zeros((K1, NF), f32)
    rep1 = np.zeros((26, K2), f32)
    for c in range(PK):
        w1bd[c * NF : (c + 1) * NF, c * 26 : (c + 1) * 26] = w1t
        w2bd[c * 26 : (c + 1) * 26, c * NF : (c + 1) * NF] = w2t
        sum1[c * 26 : (c + 1) * 26, :] = np.eye(26, dtype=f32)
        sum2[c * NF : (c + 1) * NF, :] = np.eye(NF, dtype=f32)
        rep1[:, c * 26 : (c + 1) * 26] = np.eye(26, dtype=f32)
    put("W1BD", w1bd)
    put("W2BD", w2bd)
    put("SUM1", sum1)
    put("SUM2", sum2)
    put("REP1", rep1)
    put("W3T", np.asarray(inputs["W3"], f32).T)
    put("W4T", np.asarray(inputs["W4"], f32).T)
    put("W5T", np.asarray(inputs["W5"], f32).T)
    put("KC", kcv)
    put("B1R", np.tile(np.asarray(inputs["b1"], f32), PK))
    put("B2R", np.tile(np.asarray(inputs["b2"], f32), PK))
    put("B3", inputs["b3"]); put("B4", inputs["b4"]); put("B5", inputs["b5"])
    put("G1", inputs["g1"]); put("BE1", inputs["beta1"])
    put("G2", inputs["g2"]); put("BE2", inputs["beta2"])
    d36 = np.zeros((3, 6), f32)
    for m in range(6):
        d36[m % 3, m] = 1.0
    put("D36", d36)
    # scc rows: [s0, s1, s2, c0, c1, c2]
    s1m = np.zeros((6, 6), f32)
    for m, k in ((1, 3), (2, 4), (3, 0), (4, 0), (5, 3)):
        s1m[k, m] = 1.0
    put("S1", s1m)
    s2m = np.zeros((6, 6), f32)
    for m, k in ((3, 1), (4, 2), (5, 2)):
        s2m[k, m] = 1.0
    put("S2", s2m)
    s3m = np.zeros((6, 6), f32)
    s3m[1, 5] = 1.0
    put("S3", s3m)
    put("SINB", np.array([np.pi] * 3 + [np.pi / 2] * 3, f32))
    put("M1S", np.array([0, 1, 1, 1, 1, 1], f32))
    put("M1B", np.array([1, 0, 0, 0, 0, 0], f32))
    put("M2S", np.array([0, 0, 0, 1, 1, 1], f32))
    put("M2B", np.array([1, 1, 1, 0, 0, 0], f32))
    put("M3S", np.array([0, 0, 0, 0, 0, 1], f32))
    put("M3B", np.array([1, 1, 1, 1, 1, 0], f32))
    put("EPS", np.full(K2, 1e-5, f32))
    return wpk


def _in_maps(inputs):
    x = np.ascontiguousarray(np.asarray(inputs["x"], np.float32))
    wpk = _wpack(inputs)
    maps = []
    for c in range(NCORES):
        xr = np.roll(x, -c * SH, axis=0)
        # packed layout: xS[13*cc + f, 512*j + n] = xr[512*(PK*j + cc) + n, f]
        xs = xr.reshape(NJ, PK, CH, NF).transpose(1, 3, 0, 2).reshape(K1, NJ * CH)
        maps.append({"xS": np.ascontiguousarray(xs), "wp": wpk})
    return maps


def run_spmd(inputs, **kw):
    from concourse import bass_utils

    nc = _build_nc()
    res = bass_utils.run_bass_kernel_spmd(nc, _in_maps(inputs), list(range(NCORES)), **kw)
    out = np.concatenate([res.results[c]["outT"].T for c in range(NCORES)], axis=0)
    return out.astype(np.float32), res


def kernel(**inputs):
    return run_spmd(inputs)[0]


if __name__ == "__main__":
    print("built nc ok:", _build_nc() is not None)
